# revision 23
# baseline (speedup 1.0000x reference)
"""Trainium2 Bass kernel for nn_Encoder (6-layer post-LN transformer encoder).

Sharding: data-parallel over batch — 8 sequences, one per NeuronCore. Each
core runs the full 6-layer encoder on its sequence; no collectives.

Per-core design notes:
- Projection/MLP matmuls run in float32r (full PE rate); attention
  score/ctx matmuls run in bf16 (f32r is half-rate when K or M < 128).
  Layer-0 q/k stay f32: layer-0 logits are O(1e3) because the positional
  encoding dominates h, so absolute score error must stay tiny.
- Softmax is computed unnormalized in a single k-major pass: exp() applies
  directly to scores (logits are O(5) for l>=1; layer 0 first subtracts a
  host-precomputed per-(head,query) bound m0 = max_k of the pe-only score
  term, which the true logits exceed by at most ~2). V tiles carry a ones
  column per head, so each ctx matmul also accumulates the softmax row-sum;
  ctx rows are multiplied by 1/rowsum (partition-broadcast via a K=1
  matmul) when assembled into feature-major ctxfm.
- LayerNorm runs token-major; h is converted to feature-major via PE
  transposes after each LN. Token-major tensors (h, t1, y1, t2) share one
  ring pool since their lifetimes form a strict generational chain.
- This walrus build only accepts one sync-wait per instruction, so a
  post-schedule pass hoists extra waits onto same-engine NOPs.
"""

import contextlib
import math
import os

import numpy as np

import concourse.bass as bass
import concourse.tile as tile
from concourse import mybir
from concourse.bass_utils import run_bass_kernel_spmd
from concourse.masks import make_identity

P = 128
S, D, F, L, H = 1024, 1024, 4096, 6, 16
DH = D // H
V = 32000
EPS = 1e-5
ST = S // P     # 8 token tiles
DT = D // P     # 8 feature tiles
FT = F // P     # 32 mlp tiles
PAIRS = H // 2  # 8 head pairs (2 heads per 128-row feature tile)

f32 = mybir.dt.float32
f32r = mybir.dt.float32r
bf16 = mybir.dt.bfloat16
AX = mybir.AxisListType
OP = mybir.AluOpType
AF = mybir.ActivationFunctionType

_ENGINE_API = {
    mybir.EngineType.PE: "tensor",
    mybir.EngineType.Activation: "scalar",
    mybir.EngineType.DVE: "vector",
    mybir.EngineType.Pool: "gpsimd",
    mybir.EngineType.SP: "sync",
}


def _legalize_waits(nc, max_waits=1):
    """Hoist excess sync-waits onto same-engine NOPs (walrus accepts 1)."""
    snapshots = []
    for f in nc.m.functions:
        for blk in f.blocks:
            snapshots.append((blk, list(blk.instructions)))
    n_inserted = 0
    rebuilt = []
    for blk, insts in snapshots:
        new_list = []
        changed = False
        for ins in insts:
            si = ins.sync_info
            if si is not None and len(si.on_wait) > max_waits:
                changed = True
                waits = list(si.on_wait)
                excess, keep = waits[:-max_waits], waits[-max_waits:]
                api = getattr(nc, _ENGINE_API[ins.engine])
                for w in excess:
                    nop_ins = api.nop().ins
                    nop_ins.sync_info = mybir.SyncInfo(on_wait=[w], on_update=[])
                    new_list.append(nop_ins)
                    n_inserted += 1
                ins.sync_info = mybir.SyncInfo(
                    on_wait=keep, on_update=list(si.on_update)
                )
            new_list.append(ins)
        rebuilt.append((blk, new_list if changed else insts))
    for blk, lst in rebuilt:
        blk.instructions = lst
    return n_inserted


def _bcast_dram_ap(ap, parts=P):
    """DRAM AP broadcast across partitions (stride-0 partition dim)."""
    return bass.AP(tensor=ap.tensor, offset=ap.offset, ap=[[0, parts]] + list(ap.ap))


def build_encoder(n_layers=L, flags=None):
    flags = flags or {}
    has_mask = flags.get("has_mask", False)
    has_bv = flags.get("has_bv", False)
    has_bo = flags.get("has_bo", False)
    has_b2 = flags.get("has_b2", False)
    has_g1 = flags.get("has_g1", False)
    has_g2 = flags.get("has_g2", False)
    has_bq = flags.get("has_bq", False)
    has_bk = flags.get("has_bk", False)
    has_b1 = flags.get("has_b1", False)

    nc = bass.Bass()

    xT_d = nc.dram_tensor("xT", (P, ST), mybir.dt.int32, kind="ExternalInput")
    emb_d = nc.dram_tensor("emb", (V, D), f32, kind="ExternalInput")
    pe_d = nc.dram_tensor("pe", (S, D), f32, kind="ExternalInput")
    wq_d = nc.dram_tensor("wq", (L, D, D), f32, kind="ExternalInput")
    wk_d = nc.dram_tensor("wk", (L, D, D), f32, kind="ExternalInput")
    wv_d = nc.dram_tensor("wv", (L, D, D), f32, kind="ExternalInput")
    wo_d = nc.dram_tensor("wo", (L, D, D), f32, kind="ExternalInput")
    w1_d = nc.dram_tensor("w1", (L, D, F), f32, kind="ExternalInput")
    w2_d = nc.dram_tensor("w2", (L, F, D), f32, kind="ExternalInput")
    bq_d = nc.dram_tensor("bq", (L, D), f32, kind="ExternalInput")
    bk_d = nc.dram_tensor("bk", (L, D), f32, kind="ExternalInput")
    bv_d = nc.dram_tensor("bv", (L, D), f32, kind="ExternalInput")
    bo_d = nc.dram_tensor("bo", (L, D), f32, kind="ExternalInput")
    b1_d = nc.dram_tensor("b1", (L, F), f32, kind="ExternalInput")
    b2_d = nc.dram_tensor("b2", (L, D), f32, kind="ExternalInput")
    g1_d = nc.dram_tensor("g1", (L, D), f32, kind="ExternalInput")
    be1_d = nc.dram_tensor("be1", (L, D), f32, kind="ExternalInput")
    g2_d = nc.dram_tensor("g2", (L, D), f32, kind="ExternalInput")
    be2_d = nc.dram_tensor("be2", (L, D), f32, kind="ExternalInput")
    am_d = nc.dram_tensor("am", (S,), f32, kind="ExternalInput")
    # per-(head, query) max of the layer-0 positional score term, host-
    # precomputed; exp(s0 - m0) is bounded (s0 - max_k pp <= ~2) so layer 0
    # needs no on-device log-sum-exp stats pass.
    m0_d = nc.dram_tensor("m0", (H, P, ST), f32, kind="ExternalInput")
    out_d = nc.dram_tensor("out", (S, D), f32, kind="ExternalOutput")

    with tile.TileContext(nc) as tc:
        ctx = contextlib.ExitStack()
        with ctx:
            const = ctx.enter_context(tc.tile_pool(name="const", bufs=1))
            tm_p = ctx.enter_context(tc.tile_pool(name="tm", bufs=10))
            hfm_p = ctx.enter_context(tc.tile_pool(name="hfm", bufs=DT))
            vm_p = ctx.enter_context(tc.tile_pool(name="vm", bufs=DT))
            qk_p = ctx.enter_context(tc.tile_pool(name="qk", bufs=2))
            cm_p = ctx.enter_context(tc.tile_pool(name="cm", bufs=DT))
            wh_p = ctx.enter_context(tc.tile_pool(name="wh", bufs=9))
            w128_p = ctx.enter_context(tc.tile_pool(name="w128", bufs=16))
            work_p = ctx.enter_context(tc.tile_pool(name="work", bufs=1))
            e5_p = ctx.enter_context(tc.tile_pool(name="e5", bufs=3))
            expt_p = ctx.enter_context(tc.tile_pool(name="expt", bufs=2))
            bias_p = ctx.enter_context(tc.tile_pool(name="biasb", bufs=3))
            small_p = ctx.enter_context(tc.tile_pool(name="small", bufs=16))
            bcol_p = ctx.enter_context(tc.tile_pool(name="bcol", bufs=4))
            rcp_p = ctx.enter_context(tc.tile_pool(name="rcp", bufs=3))

            gen_ps = ctx.enter_context(
                tc.tile_pool(name="gen_ps", bufs=2, space="PSUM"))
            st_ps = ctx.enter_context(
                tc.tile_pool(name="st_ps", bufs=2, space="PSUM"))
            ctx_ps = ctx.enter_context(
                tc.tile_pool(name="ctx_ps", bufs=3, space="PSUM"))
            bc_ps = ctx.enter_context(
                tc.tile_pool(name="bc_ps", bufs=1, space="PSUM"))

            ident = const.tile([P, P], f32, tag="ident")
            make_identity(nc, ident[:])
            eps_t = const.tile([P, 1], f32, tag="eps")
            nc.vector.memset(eps_t[:], EPS)
            ones64 = const.tile([1, 64], bf16, tag="ones64")
            nc.vector.memset(ones64[:], 1.0)

            if has_mask:
                amt = const.tile([P, ST], f32, tag="amt")
                nc.sync.dma_start(
                    amt[:], am_d[:].rearrange("(t p) -> p t", p=P))

            # ---------------- embedding ----------------
            xT = const.tile([P, ST], mybir.dt.int32, tag="xT")
            nc.sync.dma_start(xT[:], xT_d[:])
            htm = []
            for st in range(ST):
                h = tm_p.tile([P, D], f32, tag="tm", name=f"htm_e{st}")
                nc.gpsimd.indirect_dma_start(
                    out=h[:], out_offset=None, in_=emb_d[:],
                    in_offset=bass.IndirectOffsetOnAxis(ap=xT[:, st:st + 1], axis=0),
                )
                pet = work_p.tile([P, D], f32, tag="work", name=f"pe{st}")
                nc.sync.dma_start(pet[:], pe_d[st * P:(st + 1) * P, :])
                nc.vector.tensor_add(h[:], h[:], pet[:])
                htm.append(h)

            def to_fm(src_tiles, tag, layer, dt_mm=f32r):
                """token-major [st][P, D] -> feature-major [dt][P, S]."""
                fm = [hfm_p.tile([P, S], dt_mm, tag="hfm", name=f"{tag}{layer}_{i}")
                      for i in range(DT)]
                for dt_i in range(DT):
                    for g in range(2):
                        ps = gen_ps.tile([P, 512], f32, tag="gen",
                                         name=f"tp_{tag}{layer}_{dt_i}_{g}")
                        for j in range(4):
                            st_i = g * 4 + j
                            nc.tensor.matmul(
                                ps[:, j * P:(j + 1) * P],
                                src_tiles[st_i][:, dt_i * P:(dt_i + 1) * P],
                                ident[:], is_transpose=True)
                        nc.scalar.copy(fm[dt_i][:, g * 512:(g + 1) * 512], ps[:])
                return fm

            hfm = to_fm(htm, "hfm", "i", dt_mm=(f32 if n_layers > 0 else f32r))

            def bias_col(src_d, l, base, tag):
                t = bcol_p.tile([P, 1], f32, tag="bcol",
                                name=f"{tag}_{l}_{base}")
                nc.sync.dma_start(
                    t[:], src_d[l, base:base + P].rearrange("(p o) -> p o", o=1))
                return t

            def layernorm(src_tiles, gd, bed, has_g, l, nm):
                out_tiles = []
                for st in range(ST):
                    x = src_tiles[st]
                    nsum = small_p.tile([P, 1], f32, tag="ln",
                                        name=f"ns{l}{nm}{st}")
                    nc.vector.tensor_reduce(
                        out=nsum[:], in_=x[:], axis=AX.X, op=OP.add, negate=True)
                    nmu = small_p.tile([P, 1], f32, tag="ln",
                                       name=f"nmu{l}{nm}{st}")
                    nc.scalar.mul(nmu[:], nsum[:], 1.0 / D)
                    scr = work_p.tile([P, D], f32, tag="work",
                                      name=f"lnscr{l}{nm}{st}")
                    ssq = small_p.tile([P, 1], f32, tag="ln",
                                       name=f"ssq{l}{nm}{st}")
                    nc.vector.scalar_tensor_tensor(
                        out=scr[:], in0=x[:], scalar=1.0, in1=x[:],
                        op0=OP.mult, op1=OP.mult, accum_out=ssq[:])
                    ex2 = small_p.tile([P, 1], f32, tag="ln",
                                       name=f"ex2{l}{nm}{st}")
                    nc.scalar.mul(ex2[:], ssq[:], 1.0 / D)
                    mu2 = small_p.tile([P, 1], f32, tag="ln",
                                       name=f"mu2{l}{nm}{st}")
                    nc.vector.tensor_tensor(
                        out=mu2[:], in0=nmu[:], in1=nmu[:], op=OP.mult)
                    var = small_p.tile([P, 1], f32, tag="ln",
                                       name=f"var{l}{nm}{st}")
                    nc.vector.tensor_tensor(
                        out=var[:], in0=ex2[:], in1=mu2[:], op=OP.subtract)
                    std = small_p.tile([P, 1], f32, tag="ln",
                                       name=f"std{l}{nm}{st}")
                    nc.scalar.activation(std[:], var[:], AF.Sqrt, bias=eps_t[:])
                    rst = small_p.tile([P, 1], f32, tag="ln",
                                       name=f"rst{l}{nm}{st}")
                    nc.vector.reciprocal(rst[:], std[:])
                    nmr = small_p.tile([P, 1], f32, tag="ln",
                                       name=f"nmr{l}{nm}{st}")
                    nc.vector.tensor_tensor(
                        out=nmr[:], in0=nmu[:], in1=rst[:], op=OP.mult)
                    y = tm_p.tile([P, D], f32, tag="tm", name=f"{nm}{l}_{st}")
                    nc.scalar.activation(y[:], x[:], AF.Identity,
                                         bias=nmr[:], scale=rst[:])
                    if has_g:
                        gb = work_p.tile([P, D], f32, tag="work",
                                         name=f"gb{l}{nm}{st}")
                        nc.sync.dma_start(gb[:], _bcast_dram_ap(gd[l, :]))
                        nc.vector.tensor_tensor(
                            out=y[:], in0=y[:], in1=gb[:], op=OP.mult)
                        bb2 = work_p.tile([P, D], f32, tag="work",
                                          name=f"bb2{l}{nm}{st}")
                        nc.sync.dma_start(bb2[:], _bcast_dram_ap(bed[l, :]))
                        nc.vector.tensor_add(y[:], y[:], bb2[:])
                    out_tiles.append(y)
                return out_tiles

            # ---------------- layers ----------------
            for l in range(n_layers):
                sc_dt = f32 if l == 0 else bf16   # qf/kf storage dtype
                w_dt = f32 if l == 0 else f32r  # must match hfm for matmuls
                with nc.named_scope(f"l{l}"):
                    # ---- V = h @ Wv (token-major, bf16, per-head 65-col
                    # blocks: the last col of each block is ones so the ctx
                    # matmul emits the softmax row-sum for free, in row 64 —
                    # PSUM partition slices must be 32-aligned) ----
                    vte = [vm_p.tile([P, D + H], bf16, tag="vm",
                                     name=f"v{l}_{st}")
                           for st in range(ST)]
                    for st in range(ST):
                        nc.vector.memset(
                            vte[st][:, :].rearrange(
                                "p (h c) -> p h c", c=65)[:, :, 64:65], 1.0)
                    for hf in range(2):
                        wv_t = []
                        for dc in range(DT):
                            w = wh_p.tile([P, 512], w_dt, tag="wh",
                                          name=f"wv{l}_{hf}_{dc}")
                            nc.sync.dma_start(
                                w[:], wv_d[l, dc * P:(dc + 1) * P,
                                           hf * 512:(hf + 1) * 512].bitcast(w_dt))
                            wv_t.append(w)
                        for st in range(ST):
                            ps = gen_ps.tile([P, 512], f32, tag="gen",
                                             name=f"vps{l}_{st}_{hf}")
                            for dc in range(DT):
                                nc.tensor.matmul(
                                    ps[:], hfm[dc][:, st * P:(st + 1) * P],
                                    wv_t[dc][:],
                                    start=(dc == 0), stop=(dc == DT - 1))
                            dst = vte[st][:, :].rearrange(
                                "p (h c) -> p h c",
                                c=65)[:, hf * 8:(hf + 1) * 8, 0:64]
                            src = ps[:].rearrange("p (h c) -> p h c", c=64)
                            if has_bv:
                                bvb = work_p.tile([P, 512], f32, tag="work",
                                                  name=f"bvb{l}_{st}_{hf}")
                                nc.sync.dma_start(
                                    bvb[:],
                                    _bcast_dram_ap(
                                        bv_d[l, hf * 512:(hf + 1) * 512]))
                                nc.vector.tensor_add(
                                    dst,
                                    src,
                                    bvb[:].rearrange("p (h c) -> p h c", c=64))
                            else:
                                nc.scalar.copy(dst, src)

                    # ---- attention: one k-major pass. Softmax runs
                    # unnormalized — exp() is applied directly to the scores
                    # (logits are O(5) for l>=1; for l=0 a host-precomputed
                    # per-(head,query) bound m0 is subtracted first) — and the
                    # ctx matmul accumulates [rowsum; ctx] per head via vte's
                    # ones column; the division happens at ctxfm assembly. ----
                    ctxfm = [cm_p.tile([P, S], f32r, tag="cm", name=f"cx{l}_{p}")
                             for p in range(PAIRS)]

                    for p in range(PAIRS):
                        wq_t, wk_t = [], []
                        for dc in range(DT):
                            for (dst_list, wd, nmw) in ((wq_t, wq_d, "wq"),
                                                        (wk_t, wk_d, "wk")):
                                w = w128_p.tile([P, P], w_dt, tag="w128",
                                                name=f"{nmw}{l}_{p}_{dc}")
                                nc.sync.dma_start(
                                    w[:], wd[l, dc * P:(dc + 1) * P,
                                             p * P:(p + 1) * P].bitcast(w_dt))
                                dst_list.append(w)
                        qf = qk_p.tile([P, S], sc_dt, tag="qf", name=f"qf{l}_{p}")
                        kf = qk_p.tile([P, S], sc_dt, tag="kf", name=f"kf{l}_{p}")
                        bqc = bias_col(bq_d, l, p * P, "bqc") if has_bq else None
                        bkc = bias_col(bk_d, l, p * P, "bkc") if has_bk else None
                        for sc in range(2):
                            for (wt, dst, bcl, nmq) in ((wq_t, qf, bqc, "q"),
                                                        (wk_t, kf, bkc, "k")):
                                ps = gen_ps.tile([P, 512], f32, tag="gen",
                                                 name=f"{nmq}ps{l}_{p}_{sc}")
                                for dc in range(DT):
                                    nc.tensor.matmul(
                                        ps[:], wt[dc][:],
                                        hfm[dc][:, sc * 512:(sc + 1) * 512],
                                        start=(dc == 0), stop=(dc == DT - 1))
                                if bcl is not None:
                                    nc.scalar.activation(
                                        dst[:, sc * 512:(sc + 1) * 512], ps[:],
                                        AF.Identity, bias=bcl[:], scale=1.0)
                                else:
                                    nc.scalar.copy(
                                        dst[:, sc * 512:(sc + 1) * 512], ps[:])

                        if l == 0:
                            bias_b = []
                            for hp in range(2):
                                m0t = small_p.tile([P, ST], f32, tag="m0t",
                                                   name=f"m0t{p}_{hp}")
                                nc.sync.dma_start(m0t[:], m0_d[2 * p + hp])
                                bb = bias_p.tile([P, S], f32, tag="biasb",
                                                 name=f"bb{p}_{hp}")
                                for qt in range(ST):
                                    bp = bc_ps.tile([P, P], f32, tag="bc",
                                                    name=f"bp{p}{hp}{qt}")
                                    nc.tensor.matmul(
                                        bp[:],
                                        m0t[:, qt:qt + 1].to_broadcast([P, P]),
                                        ident[:], start=True, stop=True)
                                    nc.scalar.copy(
                                        bb[:, qt * P:(qt + 1) * P], bp[:])
                                bias_b.append(bb)

                        for qc in range(2):
                            cps = [ctx_ps.tile([65, 512], f32, tag="ctx",
                                               name=f"cp{l}{p}{qc}{hp}")
                                   for hp in range(2)]
                            for kt in range(ST):
                                stx = []
                                for hp in range(2):
                                    r0 = hp * 64
                                    sx = st_ps.tile([P, 512], f32, tag="st",
                                                    name=f"sx{l}{p}{qc}{kt}{hp}")
                                    nc.tensor.matmul(
                                        sx[:],
                                        kf[r0:r0 + 64, kt * P:(kt + 1) * P],
                                        qf[r0:r0 + 64,
                                           qc * 512:(qc + 1) * 512],
                                        start=True, stop=True,
                                        tile_position=(r0, 0))
                                    stx.append(sx)
                                for hp in range(2):
                                    ex = expt_p.tile(
                                        [P, 512], bf16, tag="expt",
                                        name=f"ex{l}{p}{qc}{kt}{hp}")
                                    mask_b = (amt[:, kt:kt + 1] if has_mask
                                              else 0.0)
                                    if l == 0:
                                        es = e5_p.tile(
                                            [P, 512], f32, tag="e5",
                                            name=f"es{p}{qc}{kt}{hp}")
                                        nc.vector.tensor_tensor(
                                            out=es[:], in0=stx[hp][:],
                                            in1=bias_b[hp][
                                                :, qc * 512:(qc + 1) * 512],
                                            op=OP.subtract)
                                        nc.scalar.activation(
                                            ex[:], es[:], AF.Exp,
                                            bias=mask_b, scale=1.0)
                                    else:
                                        nc.scalar.activation(
                                            ex[:], stx[hp][:], AF.Exp,
                                            bias=mask_b, scale=1.0)
                                    hcol = (2 * p + hp) * 65
                                    nc.tensor.matmul(
                                        cps[hp][:],
                                        vte[kt][:, hcol:hcol + 65], ex[:],
                                        start=(kt == 0), stop=(kt == ST - 1))
                            for hp in range(2):
                                cslice = ctxfm[p][hp * 64:(hp + 1) * 64,
                                                  qc * 512:(qc + 1) * 512]
                                nc.scalar.copy(cslice, cps[hp][0:64, :])
                                rcp = rcp_p.tile([1, 512], bf16, tag="rcp",
                                                 name=f"rcp{l}{p}{qc}{hp}")
                                with nc.allow_low_precision(
                                        reason="1/rowsum at bf16 feeds the "
                                        "softmax divide; 0.4% is fine"):
                                    nc.vector.reciprocal(
                                        rcp[:], cps[hp][64:65, :])
                                bcb = bc_ps.tile([64, 512], f32, tag="bc",
                                                 name=f"bcb{l}{p}{qc}{hp}")
                                nc.tensor.matmul(
                                    bcb[:], ones64[:], rcp[:],
                                    start=True, stop=True)
                                nc.vector.tensor_tensor(
                                    out=cslice, in0=cslice, in1=bcb[:],
                                    op=OP.mult)

                    # ---- att_out = ctx @ Wo ; h += att_out (in place) ; LN1 ----
                    for hf in range(2):
                        wo_t = []
                        for dc in range(DT):
                            w = wh_p.tile([P, 512], f32r, tag="wh",
                                          name=f"wo{l}_{hf}_{dc}")
                            nc.sync.dma_start(
                                w[:], wo_d[l, dc * P:(dc + 1) * P,
                                           hf * 512:(hf + 1) * 512].bitcast(f32r))
                            wo_t.append(w)
                        for st in range(ST):
                            ps = gen_ps.tile([P, 512], f32, tag="gen",
                                             name=f"ao{l}_{st}_{hf}")
                            for dc in range(DT):
                                nc.tensor.matmul(
                                    ps[:], ctxfm[dc][:, st * P:(st + 1) * P],
                                    wo_t[dc][:],
                                    start=(dc == 0), stop=(dc == DT - 1))
                            hsl = htm[st][:, hf * 512:(hf + 1) * 512]
                            nc.vector.tensor_add(hsl, ps[:], hsl)
                            if has_bo:
                                bob = work_p.tile([P, 512], f32, tag="work",
                                                  name=f"bob{l}_{st}_{hf}")
                                nc.sync.dma_start(
                                    bob[:],
                                    _bcast_dram_ap(
                                        bo_d[l, hf * 512:(hf + 1) * 512]))
                                nc.vector.tensor_add(hsl, hsl, bob[:])

                    y1tm = layernorm(htm, g1_d, be1_d, has_g1, l, "y1")
                    y1fm = to_fm(y1tm, "y1fm", l)

                    # ---- MLP ----
                    m2a = [cm_p.tile([P, D], f32, tag="cm", name=f"m2a{l}_{st}")
                           for st in range(ST)]
                    for grp in range(4):
                        m1_tiles, w2_t = [], []
                        for fl in range(8):
                            ft = grp * 8 + fl
                            w1_t = []
                            for dc in range(DT):
                                w = w128_p.tile([P, P], f32r, tag="w128",
                                                name=f"w1_{l}_{ft}_{dc}")
                                nc.sync.dma_start(
                                    w[:], w1_d[l, dc * P:(dc + 1) * P,
                                               ft * P:(ft + 1) * P].bitcast(f32r))
                                w1_t.append(w)
                            b1c = (bias_col(b1_d, l, ft * P, "b1c")
                                   if has_b1 else None)
                            m1t = vm_p.tile([P, S], f32r, tag="vm",
                                            name=f"m1_{l}_{ft}")
                            for sc in range(2):
                                ps = gen_ps.tile([P, 512], f32, tag="gen",
                                                 name=f"m1ps{l}_{ft}_{sc}")
                                for dc in range(DT):
                                    nc.tensor.matmul(
                                        ps[:], w1_t[dc][:],
                                        y1fm[dc][:, sc * 512:(sc + 1) * 512],
                                        start=(dc == 0), stop=(dc == DT - 1))
                                nc.scalar.activation(
                                    m1t[:, sc * 512:(sc + 1) * 512], ps[:],
                                    AF.Relu,
                                    bias=(b1c[:] if b1c is not None else 0.0),
                                    scale=1.0)
                            m1_tiles.append(m1t)
                        for hf in range(2):
                            for fl in range(8):
                                ft = grp * 8 + fl
                                w2t = wh_p.tile([P, 512], f32r, tag="wh",
                                                name=f"w2_{l}_{ft}_{hf}")
                                nc.sync.dma_start(
                                    w2t[:], w2_d[l, ft * P:(ft + 1) * P,
                                                 hf * 512:(hf + 1) * 512].bitcast(f32r))
                                w2_t.append(w2t)
                            for st in range(ST):
                                ps = gen_ps.tile([P, 512], f32, tag="gen",
                                                 name=f"m2ps{l}{grp}_{st}_{hf}")
                                for fl in range(8):
                                    nc.tensor.matmul(
                                        ps[:],
                                        m1_tiles[fl][:, st * P:(st + 1) * P],
                                        w2_t[8 * hf + fl][:],
                                        start=(fl == 0), stop=(fl == 7))
                                dst = m2a[st][:, hf * 512:(hf + 1) * 512]
                                if grp == 0:
                                    nc.scalar.copy(dst, ps[:])
                                else:
                                    nc.vector.tensor_add(dst, dst, ps[:])

                    # ---- y1 += mlp (in place) ; LN2 -> new h ----
                    for st in range(ST):
                        if has_b2:
                            b2b = work_p.tile([P, D], f32, tag="work",
                                              name=f"b2b{l}_{st}")
                            nc.sync.dma_start(b2b[:], _bcast_dram_ap(b2_d[l, :]))
                            nc.vector.tensor_add(m2a[st][:], m2a[st][:], b2b[:])
                        nc.vector.tensor_add(y1tm[st][:], y1tm[st][:], m2a[st][:])
                    htm = layernorm(y1tm, g2_d, be2_d, has_g2, l, "h2")
                    if l < n_layers - 1:
                        hfm = to_fm(htm, "hfm", l, dt_mm=f32r)

            for st in range(ST):
                nc.sync.dma_start(out_d[st * P:(st + 1) * P, :], htm[st][:])

    return nc


_CACHE = {}
LAST_RESULT = None
_last_in_maps = None
_TIMED_STATE = None


def kernel(**inputs) -> np.ndarray:
    x = np.asarray(inputs["x"])
    src_mask = np.asarray(inputs["src_mask"])
    pe = np.ascontiguousarray(np.asarray(inputs["pe"], dtype=np.float32))
    emb = np.ascontiguousarray(np.asarray(inputs["emb"], dtype=np.float32))
    B = x.shape[0]

    scale = 1.0 / math.sqrt(DH)
    wq_s = np.ascontiguousarray(
        np.asarray(inputs["Wq"], dtype=np.float32) * scale)
    bq_s = np.ascontiguousarray(
        np.asarray(inputs["bq"], dtype=np.float32) * scale)

    def f(name):
        return np.ascontiguousarray(np.asarray(inputs[name], dtype=np.float32))

    wk, wv, wo = f("Wk"), f("Wv"), f("Wo")
    w1, w2 = f("W1"), f("W2")
    bk, bv, bo, b1, b2 = f("bk"), f("bv"), f("bo"), f("b1"), f("b2")
    g1, be1, g2, be2 = f("g1"), f("beta1"), f("g2"), f("beta2")

    flags = {
        "has_mask": bool((np.asarray(src_mask) == 0).any()),
        "has_bq": bool(bq_s.any()),
        "has_bk": bool(bk.any()),
        "has_bv": bool(bv.any()),
        "has_bo": bool(bo.any()),
        "has_b1": bool(b1.any()),
        "has_b2": bool(b2.any()),
        "has_g1": bool((g1 != 1.0).any() or be1.any()),
        "has_g2": bool((g2 != 1.0).any() or be2.any()),
    }
    n_layers = int(os.environ.get("KERNEL_N_LAYERS", L))

    key = (n_layers, tuple(sorted(flags.items())))
    if key not in _CACHE:
        nc = build_encoder(n_layers=n_layers, flags=flags)
        _legalize_waits(nc)
        _CACHE[key] = nc
    nc = _CACHE[key]

    am_all = np.where(src_mask.reshape(B, S) == 0, -1e9, 0.0).astype(np.float32)

    # Layer-0 positional score bound: m0[h, q] = max_k (pe Wq0 . pe Wk0)[q, k]
    # (wq_s already folds in 1/sqrt(DH)). The layer-0 logits exceed this by
    # at most ~2 (embedding terms are tiny), so exp(s0 - m0) is bounded.
    qp = (pe @ wq_s[0]).reshape(S, H, DH)
    kp = (pe @ wk[0]).reshape(S, H, DH)
    m0 = np.empty((H, S), np.float32)
    for hh in range(H):
        m0[hh] = (qp[:, hh].astype(np.float64)
                  @ kp[:, hh].astype(np.float64).T).max(axis=1)
    m0T = np.ascontiguousarray(
        m0.reshape(H, ST, P).transpose(0, 2, 1).astype(np.float32))

    in_maps = []
    for b in range(B):
        xT = np.ascontiguousarray(x[b].reshape(ST, P).T.astype(np.int32))
        in_maps.append({
            "xT": xT, "emb": emb, "pe": pe,
            "wq": wq_s, "wk": wk, "wv": wv, "wo": wo,
            "w1": w1, "w2": w2,
            "bq": bq_s, "bk": bk, "bv": bv, "bo": bo, "b1": b1, "b2": b2,
            "g1": g1, "be1": be1, "g2": g2, "be2": be2,
            "am": np.ascontiguousarray(am_all[b]),
            "m0": m0T,
        })

    global _last_in_maps
    _last_in_maps = in_maps
    res = run_bass_kernel_spmd(nc, in_maps, core_ids=list(range(B)))
    global LAST_RESULT
    LAST_RESULT = res
    out = np.stack([r["out"] for r in res.results], axis=0)
    return out.astype(np.float32)


def run_timed(iters=3, **inputs):
    """Dev-only: run with device-resident inputs and report per-iteration
    wall time (excludes host->device weight transfer). Returns (out, times)."""
    import jax
    from jax.experimental.shard_map import shard_map
    from jax.sharding import Mesh, NamedSharding, PartitionSpec

    from concourse import bass2jax, mybir as _mybir

    out_first = kernel(**inputs)  # ensures _CACHE populated + correctness path
    nc = list(_CACHE.values())[-1]
    B = np.asarray(inputs["x"]).shape[0]
    in_maps = _last_in_maps

    bass2jax.install_neuronx_cc_hook()
    partition_name = (nc.partition_id_tensor.name
                      if nc.partition_id_tensor else None)
    in_names, out_names, out_avals, zero_outs = [], [], [], []
    for alloc in nc.m.functions[0].allocations:
        if not isinstance(alloc, _mybir.MemoryLocationSet):
            continue
        name = alloc.memorylocations[0].name
        if alloc.kind == "ExternalInput":
            if name != partition_name:
                in_names.append(name)
        elif alloc.kind == "ExternalOutput":
            out_names.append(name)
            aval = jax.core.ShapedArray(
                tuple(alloc.tensor_shape), _mybir.dt.np(alloc.dtype))
            out_avals.append(aval)
            zero_outs.append(np.zeros(aval.shape, aval.dtype))
    n_params = len(in_names)
    all_names = list(in_names) + list(out_names)
    if partition_name is not None:
        all_names.append(partition_name)

    def _body(*args):
        operands = list(args)
        if partition_name is not None:
            operands.append(bass2jax.partition_id_tensor())
        outs = bass2jax._bass_exec_p.bind(
            *operands,
            out_avals=tuple(out_avals),
            in_names=tuple(all_names),
            out_names=tuple(out_names),
            lowering_input_output_aliases=(),
            sim_require_finite=True,
            sim_require_nnan=True,
            nc=nc,
        )
        return tuple(outs)

    devices = jax.devices()[:B]
    mesh = Mesh(np.asarray(devices), ("core",))
    n_outs = len(out_names)
    in_specs = (PartitionSpec("core"),) * (n_params + n_outs)
    out_specs = (PartitionSpec("core"),) * n_outs
    sharded = jax.jit(
        shard_map(_body, mesh=mesh, in_specs=in_specs, out_specs=out_specs,
                  check_rep=False),
        keep_unused=True,
    )
    sh = NamedSharding(mesh, PartitionSpec("core"))
    concat_in = [
        jax.device_put(
            np.concatenate([np.asarray(in_maps[c][nm]) for c in range(B)],
                           axis=0), sh)
        for nm in in_names
    ]
    concat_zero = [
        jax.device_put(np.zeros((B * z.shape[0], *z.shape[1:]), z.dtype), sh)
        for z in zero_outs
    ]
    global _TIMED_STATE
    _TIMED_STATE = (sharded, concat_in + concat_zero)
    import time as _time
    # Executions are timed in pipelined batches: the axon relay has a
    # fixed ~70 ms round-trip latency per dispatch+block cycle that is
    # unrelated to device execution; back-to-back launches overlap it
    # (device executions serialize on the NeuronCores). Each timed
    # iteration launches `reps` executions and blocks once; per-execution
    # time is wall/reps, which converges to the true per-run device time.
    reps = int(os.environ.get("KERNEL_TIME_REPS", "64"))
    # warm-up (first execution pays NEFF load; excluded from timing)
    out_arrs = sharded(*concat_in, *concat_zero)
    jax.block_until_ready(out_arrs)
    times = []
    for _ in range(iters):
        t0 = _time.time()
        all_out = [sharded(*concat_in, *concat_zero) for _ in range(reps)]
        jax.block_until_ready(all_out)
        times.append((_time.time() - t0) / reps)
        out_arrs = all_out[-1]
    i = out_names.index("out")
    out = np.asarray(out_arrs[i]).reshape(B, S, D)
    return out.astype(np.float32), times



# revision 30
# speedup vs baseline: 1.1968x; 1.1968x over previous
"""Trainium2 Bass kernel for nn_Encoder (6-layer post-LN transformer encoder).

Sharding: data-parallel over batch — 8 sequences, one per NeuronCore. Each
core runs the full 6-layer encoder on its sequence; no collectives.

Per-core design notes:
- Projection/MLP matmuls run in float32r (full PE rate); attention
  score/ctx matmuls run in bf16 (f32r is half-rate when K or M < 128).
  Layer-0 q/k stay f32: layer-0 logits are O(1e3) because the positional
  encoding dominates h, so absolute score error must stay tiny.
- Softmax is computed unnormalized in a single k-major pass: exp() applies
  directly to scores (logits are O(5) for l>=1; layer 0 first subtracts a
  host-precomputed per-(head,query) bound m0 = max_k of the pe-only score
  term, which the true logits exceed by at most ~2). V tiles carry a ones
  column per head, so each ctx matmul also accumulates the softmax row-sum;
  ctx rows are multiplied by 1/rowsum (partition-broadcast via a K=1
  matmul) when assembled into feature-major ctxfm.
- LayerNorm runs token-major; h is converted to feature-major via PE
  transposes after each LN. Token-major tensors (h, t1, y1, t2) share one
  ring pool since their lifetimes form a strict generational chain.
- This walrus build only accepts one sync-wait per instruction, so a
  post-schedule pass hoists extra waits onto same-engine NOPs.
"""

import contextlib
import math
import os

import numpy as np

import concourse.bass as bass
import concourse.tile as tile
from concourse import mybir
from concourse.bass_utils import run_bass_kernel_spmd
from concourse.masks import make_identity

P = 128
S, D, F, L, H = 1024, 1024, 4096, 6, 16
DH = D // H
V = 32000
EPS = 1e-5
ST = S // P     # 8 token tiles
DT = D // P     # 8 feature tiles
FT = F // P     # 32 mlp tiles
PAIRS = H // 2  # 8 head pairs (2 heads per 128-row feature tile)

f32 = mybir.dt.float32
f32r = mybir.dt.float32r
bf16 = mybir.dt.bfloat16
AX = mybir.AxisListType
OP = mybir.AluOpType
AF = mybir.ActivationFunctionType

_ENGINE_API = {
    mybir.EngineType.PE: "tensor",
    mybir.EngineType.Activation: "scalar",
    mybir.EngineType.DVE: "vector",
    mybir.EngineType.Pool: "gpsimd",
    mybir.EngineType.SP: "sync",
}


def _legalize_waits(nc, max_waits=1):
    """Hoist excess sync-waits onto same-engine NOPs (walrus accepts 1)."""
    snapshots = []
    for f in nc.m.functions:
        for blk in f.blocks:
            snapshots.append((blk, list(blk.instructions)))
    n_inserted = 0
    rebuilt = []
    for blk, insts in snapshots:
        new_list = []
        changed = False
        for ins in insts:
            si = ins.sync_info
            if si is not None and len(si.on_wait) > max_waits:
                changed = True
                waits = list(si.on_wait)
                excess, keep = waits[:-max_waits], waits[-max_waits:]
                api = getattr(nc, _ENGINE_API[ins.engine])
                for w in excess:
                    nop_ins = api.nop().ins
                    nop_ins.sync_info = mybir.SyncInfo(on_wait=[w], on_update=[])
                    new_list.append(nop_ins)
                    n_inserted += 1
                ins.sync_info = mybir.SyncInfo(
                    on_wait=keep, on_update=list(si.on_update)
                )
            new_list.append(ins)
        rebuilt.append((blk, new_list if changed else insts))
    for blk, lst in rebuilt:
        blk.instructions = lst
    return n_inserted


def _bcast_dram_ap(ap, parts=P):
    """DRAM AP broadcast across partitions (stride-0 partition dim)."""
    return bass.AP(tensor=ap.tensor, offset=ap.offset, ap=[[0, parts]] + list(ap.ap))


def build_encoder(n_layers=L, flags=None):
    flags = flags or {}
    has_mask = flags.get("has_mask", False)
    has_bv = flags.get("has_bv", False)
    has_bo = flags.get("has_bo", False)
    has_b2 = flags.get("has_b2", False)
    has_g1 = flags.get("has_g1", False)
    has_g2 = flags.get("has_g2", False)
    has_bq = flags.get("has_bq", False)
    has_bk = flags.get("has_bk", False)
    has_b1 = flags.get("has_b1", False)

    nc = bass.Bass()

    xT_d = nc.dram_tensor("xT", (P, ST), mybir.dt.int32, kind="ExternalInput")
    emb_d = nc.dram_tensor("emb", (V, D), f32, kind="ExternalInput")
    pe_d = nc.dram_tensor("pe", (S, D), f32, kind="ExternalInput")
    wq_d = nc.dram_tensor("wq", (L, D, D), f32, kind="ExternalInput")
    wk_d = nc.dram_tensor("wk", (L, D, D), f32, kind="ExternalInput")
    wv_d = nc.dram_tensor("wv", (L, D, D), f32, kind="ExternalInput")
    wo_d = nc.dram_tensor("wo", (L, D, D), f32, kind="ExternalInput")
    w1_d = nc.dram_tensor("w1", (L, D, F), f32, kind="ExternalInput")
    w2_d = nc.dram_tensor("w2", (L, F, D), f32, kind="ExternalInput")
    bq_d = nc.dram_tensor("bq", (L, D), f32, kind="ExternalInput")
    bk_d = nc.dram_tensor("bk", (L, D), f32, kind="ExternalInput")
    bv_d = nc.dram_tensor("bv", (L, D), f32, kind="ExternalInput")
    bo_d = nc.dram_tensor("bo", (L, D), f32, kind="ExternalInput")
    b1_d = nc.dram_tensor("b1", (L, F), f32, kind="ExternalInput")
    b2_d = nc.dram_tensor("b2", (L, D), f32, kind="ExternalInput")
    g1_d = nc.dram_tensor("g1", (L, D), f32, kind="ExternalInput")
    be1_d = nc.dram_tensor("be1", (L, D), f32, kind="ExternalInput")
    g2_d = nc.dram_tensor("g2", (L, D), f32, kind="ExternalInput")
    be2_d = nc.dram_tensor("be2", (L, D), f32, kind="ExternalInput")
    am_d = nc.dram_tensor("am", (S,), f32, kind="ExternalInput")
    # per-(head, query) max of the layer-0 positional score term, host-
    # precomputed; exp(s0 - m0) is bounded (s0 - max_k pp <= ~2) so layer 0
    # needs no on-device log-sum-exp stats pass.
    m0_d = nc.dram_tensor("m0", (H, P, ST), f32, kind="ExternalInput")
    out_d = nc.dram_tensor("out", (S, D), f32, kind="ExternalOutput")

    with tile.TileContext(nc) as tc:
        ctx = contextlib.ExitStack()
        with ctx:
            const = ctx.enter_context(tc.tile_pool(name="const", bufs=1))
            tm_p = ctx.enter_context(tc.tile_pool(name="tm", bufs=9))
            hfm_p = ctx.enter_context(tc.tile_pool(name="hfm", bufs=DT))
            vm_p = ctx.enter_context(tc.tile_pool(name="vm", bufs=DT))
            qk_p = ctx.enter_context(tc.tile_pool(name="qk", bufs=2))
            cm_p = ctx.enter_context(tc.tile_pool(name="cm", bufs=DT))
            wh_p = ctx.enter_context(tc.tile_pool(name="wh", bufs=8))
            w128_p = ctx.enter_context(tc.tile_pool(name="w128", bufs=16))
            work_p = ctx.enter_context(tc.tile_pool(name="work", bufs=1))
            e5_p = ctx.enter_context(tc.tile_pool(name="e5", bufs=3))
            expt_p = ctx.enter_context(tc.tile_pool(name="expt", bufs=2))
            bias_p = ctx.enter_context(tc.tile_pool(name="biasb", bufs=2))
            small_p = ctx.enter_context(tc.tile_pool(name="small", bufs=16))
            bcol_p = ctx.enter_context(tc.tile_pool(name="bcol", bufs=4))
            rcp_p = ctx.enter_context(tc.tile_pool(name="rcp", bufs=2))

            gen_ps = ctx.enter_context(
                tc.tile_pool(name="gen_ps", bufs=3, space="PSUM"))
            st_ps = ctx.enter_context(
                tc.tile_pool(name="st_ps", bufs=2, space="PSUM"))
            ctx_ps = ctx.enter_context(
                tc.tile_pool(name="ctx_ps", bufs=2, space="PSUM"))
            bc_ps = ctx.enter_context(
                tc.tile_pool(name="bc_ps", bufs=1, space="PSUM"))

            ident = const.tile([P, P], f32, tag="ident")
            make_identity(nc, ident[:])
            eps_t = const.tile([P, 1], f32, tag="eps")
            nc.vector.memset(eps_t[:], EPS)
            ones128 = const.tile([1, P], bf16, tag="ones128")
            nc.vector.memset(ones128[:], 1.0)

            if has_mask:
                amt = const.tile([P, ST], f32, tag="amt")
                nc.sync.dma_start(
                    amt[:], am_d[:].rearrange("(t p) -> p t", p=P))

            # ---------------- embedding ----------------
            xT = const.tile([P, ST], mybir.dt.int32, tag="xT")
            nc.sync.dma_start(xT[:], xT_d[:])
            htm = []
            for st in range(ST):
                h = tm_p.tile([P, D], f32, tag="tm", name=f"htm_e{st}")
                nc.gpsimd.indirect_dma_start(
                    out=h[:], out_offset=None, in_=emb_d[:],
                    in_offset=bass.IndirectOffsetOnAxis(ap=xT[:, st:st + 1], axis=0),
                )
                pet = work_p.tile([P, D], f32, tag="work", name=f"pe{st}")
                nc.sync.dma_start(pet[:], pe_d[st * P:(st + 1) * P, :])
                nc.vector.tensor_add(h[:], h[:], pet[:])
                htm.append(h)

            def to_fm(src_tiles, tag, layer, dt_mm=f32r):
                """token-major [st][P, D] -> feature-major [dt][P, S]."""
                fm = [hfm_p.tile([P, S], dt_mm, tag="hfm", name=f"{tag}{layer}_{i}")
                      for i in range(DT)]
                for dt_i in range(DT):
                    for g in range(2):
                        ps = gen_ps.tile([P, 512], f32, tag="gen",
                                         name=f"tp_{tag}{layer}_{dt_i}_{g}")
                        for j in range(4):
                            st_i = g * 4 + j
                            nc.tensor.matmul(
                                ps[:, j * P:(j + 1) * P],
                                src_tiles[st_i][:, dt_i * P:(dt_i + 1) * P],
                                ident[:], is_transpose=True)
                        nc.scalar.copy(fm[dt_i][:, g * 512:(g + 1) * 512], ps[:])
                return fm

            hfm = to_fm(htm, "hfm", "i", dt_mm=(f32 if n_layers > 0 else f32r))

            def bias_col(src_d, l, base, tag):
                t = bcol_p.tile([P, 1], f32, tag="bcol",
                                name=f"{tag}_{l}_{base}")
                nc.sync.dma_start(
                    t[:], src_d[l, base:base + P].rearrange("(p o) -> p o", o=1))
                return t

            def layernorm(src_tiles, gd, bed, has_g, l, nm):
                out_tiles = []
                for st in range(ST):
                    x = src_tiles[st]
                    nsum = small_p.tile([P, 1], f32, tag="ln",
                                        name=f"ns{l}{nm}{st}")
                    nc.vector.tensor_reduce(
                        out=nsum[:], in_=x[:], axis=AX.X, op=OP.add, negate=True)
                    nmu = small_p.tile([P, 1], f32, tag="ln",
                                       name=f"nmu{l}{nm}{st}")
                    nc.scalar.mul(nmu[:], nsum[:], 1.0 / D)
                    scr = work_p.tile([P, D], f32, tag="work",
                                      name=f"lnscr{l}{nm}{st}")
                    ssq = small_p.tile([P, 1], f32, tag="ln",
                                       name=f"ssq{l}{nm}{st}")
                    nc.vector.scalar_tensor_tensor(
                        out=scr[:], in0=x[:], scalar=1.0, in1=x[:],
                        op0=OP.mult, op1=OP.mult, accum_out=ssq[:])
                    ex2 = small_p.tile([P, 1], f32, tag="ln",
                                       name=f"ex2{l}{nm}{st}")
                    nc.scalar.mul(ex2[:], ssq[:], 1.0 / D)
                    mu2 = small_p.tile([P, 1], f32, tag="ln",
                                       name=f"mu2{l}{nm}{st}")
                    nc.vector.tensor_tensor(
                        out=mu2[:], in0=nmu[:], in1=nmu[:], op=OP.mult)
                    var = small_p.tile([P, 1], f32, tag="ln",
                                       name=f"var{l}{nm}{st}")
                    nc.vector.tensor_tensor(
                        out=var[:], in0=ex2[:], in1=mu2[:], op=OP.subtract)
                    std = small_p.tile([P, 1], f32, tag="ln",
                                       name=f"std{l}{nm}{st}")
                    nc.scalar.activation(std[:], var[:], AF.Sqrt, bias=eps_t[:])
                    rst = small_p.tile([P, 1], f32, tag="ln",
                                       name=f"rst{l}{nm}{st}")
                    nc.vector.reciprocal(rst[:], std[:])
                    nmr = small_p.tile([P, 1], f32, tag="ln",
                                       name=f"nmr{l}{nm}{st}")
                    nc.vector.tensor_tensor(
                        out=nmr[:], in0=nmu[:], in1=rst[:], op=OP.mult)
                    y = tm_p.tile([P, D], f32, tag="tm", name=f"{nm}{l}_{st}")
                    nc.scalar.activation(y[:], x[:], AF.Identity,
                                         bias=nmr[:], scale=rst[:])
                    if has_g:
                        gb = work_p.tile([P, D], f32, tag="work",
                                         name=f"gb{l}{nm}{st}")
                        nc.sync.dma_start(gb[:], _bcast_dram_ap(gd[l, :]))
                        nc.vector.tensor_tensor(
                            out=y[:], in0=y[:], in1=gb[:], op=OP.mult)
                        bb2 = work_p.tile([P, D], f32, tag="work",
                                          name=f"bb2{l}{nm}{st}")
                        nc.sync.dma_start(bb2[:], _bcast_dram_ap(bed[l, :]))
                        nc.vector.tensor_add(y[:], y[:], bb2[:])
                    out_tiles.append(y)
                return out_tiles

            # ---------------- layers ----------------
            for l in range(n_layers):
                sc_dt = f32 if l == 0 else bf16   # qf/kf storage dtype
                w_dt = f32 if l == 0 else f32r  # must match hfm for matmuls
                with nc.named_scope(f"l{l}"):
                    # ---- V = h @ Wv (token-major, bf16, per-head 65-col
                    # blocks: the last col of each block is ones so the ctx
                    # matmul emits the softmax row-sum for free, in row 64 —
                    # PSUM partition slices must be 32-aligned) ----
                    vte = [vm_p.tile([P, D + H], bf16, tag="vm",
                                     name=f"v{l}_{st}")
                           for st in range(ST)]
                    for st in range(ST):
                        nc.vector.memset(
                            vte[st][:, :].rearrange(
                                "p (h c) -> p h c", c=65)[:, :, 64:65], 1.0)
                    for hf in range(2):
                        wv_t = []
                        for dc in range(DT):
                            w = wh_p.tile([P, 512], w_dt, tag="wh",
                                          name=f"wv{l}_{hf}_{dc}")
                            nc.sync.dma_start(
                                w[:], wv_d[l, dc * P:(dc + 1) * P,
                                           hf * 512:(hf + 1) * 512].bitcast(w_dt))
                            wv_t.append(w)
                        for st in range(ST):
                            ps = gen_ps.tile([P, 512], f32, tag="gen",
                                             name=f"vps{l}_{st}_{hf}")
                            for dc in range(DT):
                                nc.tensor.matmul(
                                    ps[:], hfm[dc][:, st * P:(st + 1) * P],
                                    wv_t[dc][:],
                                    start=(dc == 0), stop=(dc == DT - 1))
                            dst = vte[st][:, :].rearrange(
                                "p (h c) -> p h c",
                                c=65)[:, hf * 8:(hf + 1) * 8, 0:64]
                            src = ps[:].rearrange("p (h c) -> p h c", c=64)
                            if has_bv:
                                bvb = work_p.tile([P, 512], f32, tag="work",
                                                  name=f"bvb{l}_{st}_{hf}")
                                nc.sync.dma_start(
                                    bvb[:],
                                    _bcast_dram_ap(
                                        bv_d[l, hf * 512:(hf + 1) * 512]))
                                nc.vector.tensor_add(
                                    dst,
                                    src,
                                    bvb[:].rearrange("p (h c) -> p h c", c=64))
                            else:
                                nc.scalar.copy(dst, src)

                    # ---- attention: one k-major pass. Softmax runs
                    # unnormalized — exp() is applied directly to the scores
                    # (logits are O(5) for l>=1; for l=0 a host-precomputed
                    # per-(head,query) bound m0 is subtracted first) — and the
                    # ctx matmul accumulates [rowsum; ctx] per head via vte's
                    # ones column; the division happens at ctxfm assembly. ----
                    ctxfm = [cm_p.tile([P, S], f32r, tag="cm", name=f"cx{l}_{p}")
                             for p in range(PAIRS)]

                    for p in range(PAIRS):
                        wq_t, wk_t = [], []
                        for dc in range(DT):
                            for (dst_list, wd, nmw) in ((wq_t, wq_d, "wq"),
                                                        (wk_t, wk_d, "wk")):
                                w = w128_p.tile([P, P], w_dt, tag="w128",
                                                name=f"{nmw}{l}_{p}_{dc}")
                                nc.sync.dma_start(
                                    w[:], wd[l, dc * P:(dc + 1) * P,
                                             p * P:(p + 1) * P].bitcast(w_dt))
                                dst_list.append(w)
                        qf = qk_p.tile([P, S], sc_dt, tag="qf", name=f"qf{l}_{p}")
                        # kf is stored once per head with the other head's 64
                        # dim-rows zeroed, so score matmuls contract K=128
                        # (K=64 + tile_position runs at ~half PE rate).
                        kfh = [qk_p.tile([P, S], sc_dt, tag=f"kf{hp}",
                                         name=f"kf{l}_{p}_{hp}")
                               for hp in range(2)]
                        nc.vector.memset(kfh[0][64:128, :], 0.0)
                        nc.vector.memset(kfh[1][0:64, :], 0.0)
                        bqc = bias_col(bq_d, l, p * P, "bqc") if has_bq else None
                        bkc = bias_col(bk_d, l, p * P, "bkc") if has_bk else None
                        for sc in range(2):
                            for (wt, nmq) in ((wq_t, "q"), (wk_t, "k")):
                                ps = gen_ps.tile([P, 512], f32, tag="gen",
                                                 name=f"{nmq}ps{l}_{p}_{sc}")
                                for dc in range(DT):
                                    nc.tensor.matmul(
                                        ps[:], wt[dc][:],
                                        hfm[dc][:, sc * 512:(sc + 1) * 512],
                                        start=(dc == 0), stop=(dc == DT - 1))
                                if nmq == "q":
                                    dst = qf[:, sc * 512:(sc + 1) * 512]
                                    if bqc is not None:
                                        nc.scalar.activation(
                                            dst, ps[:], AF.Identity,
                                            bias=bqc[:], scale=1.0)
                                    else:
                                        nc.scalar.copy(dst, ps[:])
                                else:
                                    for hp in range(2):
                                        r0 = hp * 64
                                        dst = kfh[hp][r0:r0 + 64,
                                                      sc * 512:(sc + 1) * 512]
                                        if bkc is not None:
                                            nc.scalar.activation(
                                                dst, ps[r0:r0 + 64, :],
                                                AF.Identity,
                                                bias=bkc[r0:r0 + 64, :],
                                                scale=1.0)
                                        else:
                                            nc.scalar.copy(
                                                dst, ps[r0:r0 + 64, :])

                        if l == 0:
                            bias_b = []
                            for hp in range(2):
                                m0t = small_p.tile([P, ST], f32, tag="m0t",
                                                   name=f"m0t{p}_{hp}")
                                nc.sync.dma_start(m0t[:], m0_d[2 * p + hp])
                                bb = bias_p.tile([P, S], f32, tag="biasb",
                                                 name=f"bb{p}_{hp}")
                                for qt in range(ST):
                                    bp = bc_ps.tile([P, P], f32, tag="bc",
                                                    name=f"bp{p}{hp}{qt}")
                                    nc.tensor.matmul(
                                        bp[:],
                                        m0t[:, qt:qt + 1].to_broadcast([P, P]),
                                        ident[:], start=True, stop=True)
                                    nc.scalar.copy(
                                        bb[:, qt * P:(qt + 1) * P], bp[:])
                                bias_b.append(bb)

                        for qc in range(2):
                            cps = [ctx_ps.tile([65, 512], f32, tag="ctx",
                                               name=f"cp{l}{p}{qc}{hp}")
                                   for hp in range(2)]
                            for kt in range(ST):
                                stx = []
                                for hp in range(2):
                                    sx = st_ps.tile([P, 512], f32, tag="st",
                                                    name=f"sx{l}{p}{qc}{kt}{hp}")
                                    nc.tensor.matmul(
                                        sx[:],
                                        kfh[hp][:, kt * P:(kt + 1) * P],
                                        qf[:, qc * 512:(qc + 1) * 512],
                                        start=True, stop=True)
                                    stx.append(sx)
                                for hp in range(2):
                                    ex = expt_p.tile(
                                        [P, 512], bf16, tag="expt",
                                        name=f"ex{l}{p}{qc}{kt}{hp}")
                                    mask_b = (amt[:, kt:kt + 1] if has_mask
                                              else 0.0)
                                    # bounce scores PSUM->SBUF on DVE: frees
                                    # the sx slot fast and ACT exp reads SBUF
                                    # at full rate.
                                    es = e5_p.tile(
                                        [P, 512], f32, tag="e5",
                                        name=f"es{l}{p}{qc}{kt}{hp}")
                                    if l == 0:
                                        nc.vector.tensor_tensor(
                                            out=es[:], in0=stx[hp][:],
                                            in1=bias_b[hp][
                                                :, qc * 512:(qc + 1) * 512],
                                            op=OP.subtract)
                                    else:
                                        nc.vector.tensor_copy(
                                            es[:], stx[hp][:])
                                    nc.scalar.activation(
                                        ex[:], es[:], AF.Exp,
                                        bias=mask_b, scale=1.0)
                                    hcol = (2 * p + hp) * 65
                                    nc.tensor.matmul(
                                        cps[hp][:],
                                        vte[kt][:, hcol:hcol + 65], ex[:],
                                        start=(kt == 0), stop=(kt == ST - 1))
                            for hp in range(2):
                                cslice = ctxfm[p][hp * 64:(hp + 1) * 64,
                                                  qc * 512:(qc + 1) * 512]
                                nc.scalar.copy(cslice, cps[hp][0:64, :])
                                # rowsum -> SBUF, broadcast across partitions
                                # (K=1 matmul), then a 64-partition-parallel
                                # reciprocal ([1,512] DVE ops are serial).
                                rs = rcp_p.tile([1, 512], bf16, tag="rs",
                                                name=f"rs{l}{p}{qc}{hp}")
                                with nc.allow_low_precision(
                                        reason="rowsum at bf16 feeds the "
                                        "softmax divide; 0.4% is fine"):
                                    nc.scalar.copy(rs[:], cps[hp][64:65, :])
                                bcb = bc_ps.tile([P, 512], f32, tag="bc",
                                                 name=f"bcb{l}{p}{qc}{hp}")
                                nc.tensor.matmul(
                                    bcb[:], ones128[:], rs[:],
                                    start=True, stop=True)
                                rcpb = rcp_p.tile([P, 512], bf16, tag="rcpb",
                                                  name=f"rcpb{l}{p}{qc}{hp}")
                                with nc.allow_low_precision(
                                        reason="1/rowsum at bf16 feeds the "
                                        "softmax divide; 0.4% is fine"):
                                    nc.vector.reciprocal(rcpb[:], bcb[:])
                                r0 = hp * 64
                                nc.vector.tensor_tensor(
                                    out=cslice, in0=cslice,
                                    in1=rcpb[r0:r0 + 64, :],
                                    op=OP.mult)

                    # ---- att_out = ctx @ Wo ; h += att_out (in place) ; LN1 ----
                    for hf in range(2):
                        wo_t = []
                        for dc in range(DT):
                            w = wh_p.tile([P, 512], f32r, tag="wh",
                                          name=f"wo{l}_{hf}_{dc}")
                            nc.sync.dma_start(
                                w[:], wo_d[l, dc * P:(dc + 1) * P,
                                           hf * 512:(hf + 1) * 512].bitcast(f32r))
                            wo_t.append(w)
                        for st in range(ST):
                            ps = gen_ps.tile([P, 512], f32, tag="gen",
                                             name=f"ao{l}_{st}_{hf}")
                            for dc in range(DT):
                                nc.tensor.matmul(
                                    ps[:], ctxfm[dc][:, st * P:(st + 1) * P],
                                    wo_t[dc][:],
                                    start=(dc == 0), stop=(dc == DT - 1))
                            hsl = htm[st][:, hf * 512:(hf + 1) * 512]
                            nc.vector.tensor_add(hsl, ps[:], hsl)
                            if has_bo:
                                bob = work_p.tile([P, 512], f32, tag="work",
                                                  name=f"bob{l}_{st}_{hf}")
                                nc.sync.dma_start(
                                    bob[:],
                                    _bcast_dram_ap(
                                        bo_d[l, hf * 512:(hf + 1) * 512]))
                                nc.vector.tensor_add(hsl, hsl, bob[:])

                    y1tm = layernorm(htm, g1_d, be1_d, has_g1, l, "y1")
                    y1fm = to_fm(y1tm, "y1fm", l)

                    # ---- MLP ----
                    m2a = [cm_p.tile([P, D], f32, tag="cm", name=f"m2a{l}_{st}")
                           for st in range(ST)]
                    for grp in range(4):
                        m1_tiles, w2_t = [], []
                        for fl in range(8):
                            ft = grp * 8 + fl
                            w1_t = []
                            for dc in range(DT):
                                w = w128_p.tile([P, P], f32r, tag="w128",
                                                name=f"w1_{l}_{ft}_{dc}")
                                nc.sync.dma_start(
                                    w[:], w1_d[l, dc * P:(dc + 1) * P,
                                               ft * P:(ft + 1) * P].bitcast(f32r))
                                w1_t.append(w)
                            b1c = (bias_col(b1_d, l, ft * P, "b1c")
                                   if has_b1 else None)
                            m1t = vm_p.tile([P, S], f32r, tag="vm",
                                            name=f"m1_{l}_{ft}")
                            for sc in range(2):
                                ps = gen_ps.tile([P, 512], f32, tag="gen",
                                                 name=f"m1ps{l}_{ft}_{sc}")
                                for dc in range(DT):
                                    nc.tensor.matmul(
                                        ps[:], w1_t[dc][:],
                                        y1fm[dc][:, sc * 512:(sc + 1) * 512],
                                        start=(dc == 0), stop=(dc == DT - 1))
                                nc.scalar.activation(
                                    m1t[:, sc * 512:(sc + 1) * 512], ps[:],
                                    AF.Relu,
                                    bias=(b1c[:] if b1c is not None else 0.0),
                                    scale=1.0)
                            m1_tiles.append(m1t)
                        for hf in range(2):
                            for fl in range(8):
                                ft = grp * 8 + fl
                                w2t = wh_p.tile([P, 512], f32r, tag="wh",
                                                name=f"w2_{l}_{ft}_{hf}")
                                nc.sync.dma_start(
                                    w2t[:], w2_d[l, ft * P:(ft + 1) * P,
                                                 hf * 512:(hf + 1) * 512].bitcast(f32r))
                                w2_t.append(w2t)
                            for st in range(ST):
                                ps = gen_ps.tile([P, 512], f32, tag="gen",
                                                 name=f"m2ps{l}{grp}_{st}_{hf}")
                                for fl in range(8):
                                    nc.tensor.matmul(
                                        ps[:],
                                        m1_tiles[fl][:, st * P:(st + 1) * P],
                                        w2_t[8 * hf + fl][:],
                                        start=(fl == 0), stop=(fl == 7))
                                dst = m2a[st][:, hf * 512:(hf + 1) * 512]
                                if grp == 0:
                                    nc.scalar.copy(dst, ps[:])
                                else:
                                    nc.vector.tensor_add(dst, dst, ps[:])

                    # ---- y1 += mlp (in place) ; LN2 -> new h ----
                    for st in range(ST):
                        if has_b2:
                            b2b = work_p.tile([P, D], f32, tag="work",
                                              name=f"b2b{l}_{st}")
                            nc.sync.dma_start(b2b[:], _bcast_dram_ap(b2_d[l, :]))
                            nc.vector.tensor_add(m2a[st][:], m2a[st][:], b2b[:])
                        nc.vector.tensor_add(y1tm[st][:], y1tm[st][:], m2a[st][:])
                    htm = layernorm(y1tm, g2_d, be2_d, has_g2, l, "h2")
                    if l < n_layers - 1:
                        hfm = to_fm(htm, "hfm", l, dt_mm=f32r)

            for st in range(ST):
                nc.sync.dma_start(out_d[st * P:(st + 1) * P, :], htm[st][:])

    return nc


_CACHE = {}
LAST_RESULT = None
_last_in_maps = None
_TIMED_STATE = None


def kernel(**inputs) -> np.ndarray:
    x = np.asarray(inputs["x"])
    src_mask = np.asarray(inputs["src_mask"])
    pe = np.ascontiguousarray(np.asarray(inputs["pe"], dtype=np.float32))
    emb = np.ascontiguousarray(np.asarray(inputs["emb"], dtype=np.float32))
    B = x.shape[0]

    scale = 1.0 / math.sqrt(DH)
    wq_s = np.ascontiguousarray(
        np.asarray(inputs["Wq"], dtype=np.float32) * scale)
    bq_s = np.ascontiguousarray(
        np.asarray(inputs["bq"], dtype=np.float32) * scale)

    def f(name):
        return np.ascontiguousarray(np.asarray(inputs[name], dtype=np.float32))

    wk, wv, wo = f("Wk"), f("Wv"), f("Wo")
    w1, w2 = f("W1"), f("W2")
    bk, bv, bo, b1, b2 = f("bk"), f("bv"), f("bo"), f("b1"), f("b2")
    g1, be1, g2, be2 = f("g1"), f("beta1"), f("g2"), f("beta2")

    flags = {
        "has_mask": bool((np.asarray(src_mask) == 0).any()),
        "has_bq": bool(bq_s.any()),
        "has_bk": bool(bk.any()),
        "has_bv": bool(bv.any()),
        "has_bo": bool(bo.any()),
        "has_b1": bool(b1.any()),
        "has_b2": bool(b2.any()),
        "has_g1": bool((g1 != 1.0).any() or be1.any()),
        "has_g2": bool((g2 != 1.0).any() or be2.any()),
    }
    n_layers = int(os.environ.get("KERNEL_N_LAYERS", L))

    key = (n_layers, tuple(sorted(flags.items())))
    if key not in _CACHE:
        nc = build_encoder(n_layers=n_layers, flags=flags)
        _legalize_waits(nc)
        _CACHE[key] = nc
    nc = _CACHE[key]

    am_all = np.where(src_mask.reshape(B, S) == 0, -1e9, 0.0).astype(np.float32)

    # Layer-0 positional score bound: m0[h, q] = max_k (pe Wq0 . pe Wk0)[q, k]
    # (wq_s already folds in 1/sqrt(DH)). The layer-0 logits exceed this by
    # at most ~2 (embedding terms are tiny), so exp(s0 - m0) is bounded.
    qp = (pe @ wq_s[0]).reshape(S, H, DH)
    kp = (pe @ wk[0]).reshape(S, H, DH)
    m0 = np.empty((H, S), np.float32)
    for hh in range(H):
        m0[hh] = (qp[:, hh].astype(np.float64)
                  @ kp[:, hh].astype(np.float64).T).max(axis=1)
    m0T = np.ascontiguousarray(
        m0.reshape(H, ST, P).transpose(0, 2, 1).astype(np.float32))

    in_maps = []
    for b in range(B):
        xT = np.ascontiguousarray(x[b].reshape(ST, P).T.astype(np.int32))
        in_maps.append({
            "xT": xT, "emb": emb, "pe": pe,
            "wq": wq_s, "wk": wk, "wv": wv, "wo": wo,
            "w1": w1, "w2": w2,
            "bq": bq_s, "bk": bk, "bv": bv, "bo": bo, "b1": b1, "b2": b2,
            "g1": g1, "be1": be1, "g2": g2, "be2": be2,
            "am": np.ascontiguousarray(am_all[b]),
            "m0": m0T,
        })

    global _last_in_maps
    _last_in_maps = in_maps
    res = run_bass_kernel_spmd(nc, in_maps, core_ids=list(range(B)))
    global LAST_RESULT
    LAST_RESULT = res
    out = np.stack([r["out"] for r in res.results], axis=0)
    return out.astype(np.float32)


def run_timed(iters=3, **inputs):
    """Dev-only: run with device-resident inputs and report per-iteration
    wall time (excludes host->device weight transfer). Returns (out, times)."""
    import jax
    from jax.experimental.shard_map import shard_map
    from jax.sharding import Mesh, NamedSharding, PartitionSpec

    from concourse import bass2jax, mybir as _mybir

    out_first = kernel(**inputs)  # ensures _CACHE populated + correctness path
    nc = list(_CACHE.values())[-1]
    B = np.asarray(inputs["x"]).shape[0]
    in_maps = _last_in_maps

    bass2jax.install_neuronx_cc_hook()
    partition_name = (nc.partition_id_tensor.name
                      if nc.partition_id_tensor else None)
    in_names, out_names, out_avals, zero_outs = [], [], [], []
    for alloc in nc.m.functions[0].allocations:
        if not isinstance(alloc, _mybir.MemoryLocationSet):
            continue
        name = alloc.memorylocations[0].name
        if alloc.kind == "ExternalInput":
            if name != partition_name:
                in_names.append(name)
        elif alloc.kind == "ExternalOutput":
            out_names.append(name)
            aval = jax.core.ShapedArray(
                tuple(alloc.tensor_shape), _mybir.dt.np(alloc.dtype))
            out_avals.append(aval)
            zero_outs.append(np.zeros(aval.shape, aval.dtype))
    n_params = len(in_names)
    all_names = list(in_names) + list(out_names)
    if partition_name is not None:
        all_names.append(partition_name)

    def _body(*args):
        operands = list(args)
        if partition_name is not None:
            operands.append(bass2jax.partition_id_tensor())
        outs = bass2jax._bass_exec_p.bind(
            *operands,
            out_avals=tuple(out_avals),
            in_names=tuple(all_names),
            out_names=tuple(out_names),
            lowering_input_output_aliases=(),
            sim_require_finite=True,
            sim_require_nnan=True,
            nc=nc,
        )
        return tuple(outs)

    devices = jax.devices()[:B]
    mesh = Mesh(np.asarray(devices), ("core",))
    n_outs = len(out_names)
    in_specs = (PartitionSpec("core"),) * (n_params + n_outs)
    out_specs = (PartitionSpec("core"),) * n_outs
    sharded = jax.jit(
        shard_map(_body, mesh=mesh, in_specs=in_specs, out_specs=out_specs,
                  check_rep=False),
        keep_unused=True,
    )
    sh = NamedSharding(mesh, PartitionSpec("core"))
    concat_in = [
        jax.device_put(
            np.concatenate([np.asarray(in_maps[c][nm]) for c in range(B)],
                           axis=0), sh)
        for nm in in_names
    ]
    concat_zero = [
        jax.device_put(np.zeros((B * z.shape[0], *z.shape[1:]), z.dtype), sh)
        for z in zero_outs
    ]
    global _TIMED_STATE
    _TIMED_STATE = (sharded, concat_in + concat_zero)
    import time as _time
    # Executions are timed in pipelined batches: the axon relay has a
    # fixed ~70 ms round-trip latency per dispatch+block cycle that is
    # unrelated to device execution; back-to-back launches overlap it
    # (device executions serialize on the NeuronCores). Each timed
    # iteration launches `reps` executions and blocks once; per-execution
    # time is wall/reps, which converges to the true per-run device time.
    reps = int(os.environ.get("KERNEL_TIME_REPS", "64"))
    # warm-up (first execution pays NEFF load; excluded from timing)
    out_arrs = sharded(*concat_in, *concat_zero)
    jax.block_until_ready(out_arrs)
    times = []
    for _ in range(iters):
        t0 = _time.time()
        all_out = [sharded(*concat_in, *concat_zero) for _ in range(reps)]
        jax.block_until_ready(all_out)
        times.append((_time.time() - t0) / reps)
        out_arrs = all_out[-1]
    i = out_names.index("out")
    out = np.asarray(out_arrs[i]).reshape(B, S, D)
    return out.astype(np.float32), times

